# revision 1
# baseline (speedup 1.0000x reference)
"""CPM3 attention kernel for 8 trn2 NeuronCores — v3.

Sharding: tensor-parallel over heads (2 heads/core x both batches).
Device computes per-core partial outputs (Wo row-sharded); host sums.

Design:
- host precomputes E = mask ? exp(position_bias) : 0 (fp16), since
  softmax(s + pb - inf*mask) uses exp(s + pb)*mask = exp(s) * E.
  Main loop per 128k x 1024(2 heads x 512q) tile: QK matmul -> Exp
  (1024 wide across two PSUM banks) -> p = e*E (fp16 DVE 2x) -> PV.
- E streams on the GpSimd DMA ring in 256KB per-k-tile pieces.
- epilogue per (q-tile, batch): denominator reciprocal broadcast via
  gpsimd.partition_broadcast (no PSUM), out-projection written in-place
  into the finished ctx PSUM tile, emission staggered across main-loop
  units so no engine FIFO ever stalls on it.
- prologue DMAs split into [128,512] quarters for DMA-queue parallelism
  (single queues sustain only ~33 GB/s).
"""

import sys

sys.path.insert(0, "/opt/trn_rl_repo")

import numpy as np
import ml_dtypes

import concourse.bass as bass
import concourse.bacc as bacc
import concourse.tile as tile
import concourse.mybir as mybir
from concourse.bass_utils import run_bass_kernel_spmd

B, L, D, H, DH = 2, 2048, 1024, 16, 64
N_CORES = 8
HPC = H // N_CORES  # heads per core = 2
QTS = 512  # q tile size
QN = L // QTS  # 4
KP = 128  # k partition tile
KN = L // KP  # 16
KTG = 4  # k tiles per DMA group
KGN = KN // KTG  # 4
DC = D // 128  # 8 contraction chunks
HVW = 2 * (DH + 1)  # 130: hv_aug columns per k-tile (2 heads x (64+ones))

F32 = mybir.dt.float32
F32R = mybir.dt.float32r
F16 = mybir.dt.float16

_CACHE: dict = {}


def _build():
    if "nc" in _CACHE:
        return _CACHE["nc"]
    nc = bacc.Bacc("TRN2", target_bir_lowering=False, debug=False, num_devices=N_CORES)

    qT = nc.dram_tensor("qT", [B, DC, 128, L], F16, kind="ExternalInput").ap()
    kvT = nc.dram_tensor("kvT", [B, DC, 128, L], F16, kind="ExternalInput").ap()
    wq = nc.dram_tensor("wq", [128, DC, 128], F16, kind="ExternalInput").ap()
    wk = nc.dram_tensor("wk", [128, DC, 128], F16, kind="ExternalInput").ap()
    wv = nc.dram_tensor("wv", [128, DC, 128], F16, kind="ExternalInput").ap()
    wo = nc.dram_tensor("wo", [128, D], F16, kind="ExternalInput").ap()
    eb = nc.dram_tensor(
        "eb", [QN, B, KGN, 128, KTG, HPC * QTS], F16, kind="ExternalInput"
    ).ap()
    identr = nc.dram_tensor("identr", [128, 128], F32R, kind="ExternalInput").ap()
    out = nc.dram_tensor("out", [B, L, D], F16, kind="ExternalOutput").ap()

    with tile.TileContext(nc) as tc:
        with (
            tc.tile_pool(name="const", bufs=1) as constp,
            tc.tile_pool(name="hq", bufs=2) as hqp,
            tc.tile_pool(name="hk", bufs=2) as hkp,
            tc.tile_pool(name="hv", bufs=2) as hvp,
            tc.tile_pool(name="stage", bufs=8) as stagep,
            tc.tile_pool(name="ep", bufs=4) as epool,
            tc.tile_pool(name="p2", bufs=3) as p2p,
            tc.tile_pool(name="p3", bufs=4) as p3p,
            tc.tile_pool(name="ctxn", bufs=2) as ctxnp,
            tc.tile_pool(name="rc", bufs=2) as rcp,
            tc.tile_pool(name="outb", bufs=3) as outp,
            tc.tile_pool(name="psum", bufs=2, space=bass.MemorySpace.PSUM) as psp,
        ):
            # ---- constants ----
            identr_t = constp.tile([128, 128], F32R, tag="identr")
            nc.sync.dma_start(identr_t[:], identr[:])
            wq_t = constp.tile([128, DC, 128], F16, tag="wq")
            nc.sync.dma_start(wq_t[:], wq[:])
            wk_t = constp.tile([128, DC, 128], F16, tag="wk")
            nc.sync.dma_start(wk_t[:], wk[:])
            wv_t = constp.tile([128, DC, 128], F16, tag="wv")
            nc.sync.dma_start(wv_t[:], wv[:])
            wo_t = constp.tile([128, D], F16, tag="wo")
            nc.sync.dma_start(wo_t[:], wo[:])

            # DMA triggers cost ~600ns on the issuing engine queue: spread the
            # prologue chunk loads across three engines' rings.
            trig = [nc.sync, nc.scalar, nc.gpsimd]
            trig_i = [0]

            def dma_split(dst, src, n):
                # split a [128, L] chunk DMA into n pieces for ring
                # parallelism (one ring sustains only ~33 GB/s), cycling
                # the trigger engine (each trigger costs ~650ns there)
                w = L // n
                for s in range(n):
                    eng = trig[trig_i[0] % 3]
                    trig_i[0] += 1
                    eng.dma_start(
                        dst[:, s * w : (s + 1) * w], src[:, s * w : (s + 1) * w]
                    )

            # ---- prologue per batch: hk+hv while kv chunks stream in,
            # then hq overlapping the q chunk stream.
            hq_sb, hk_sb, hv_sb = {}, {}, {}
            kc_pre = {}  # kvT b1 chunks prefetched during the b0 phases
            for b in range(B):
                hk2 = [
                    psp.tile([128, 1024], F32, tag="ctx", name=f"hk2_{b}_{i}")
                    for i in range(2)
                ]
                hv2 = [
                    psp.tile([128, 1024], F32, tag="sc", name=f"hv2_{b}_{i}")
                    for i in range(2)
                ]
                for dc in range(DC):
                    if (b, dc) in kc_pre:
                        kc = kc_pre[b, dc]
                    else:
                        kc = stagep.tile([128, L], F16, tag="stage")
                        dma_split(kc, kvT[b, dc], 8 if (b == 0 and dc < 2) else 2)
                    for half in range(2):
                        for qq in range(2):
                            src = kc[:, (half * 2 + qq) * QTS : (half * 2 + qq + 1) * QTS]
                            nc.tensor.matmul(
                                hk2[half][:, qq * QTS : (qq + 1) * QTS],
                                wk_t[:, dc, :],
                                src,
                                start=(dc == 0),
                                stop=(dc == DC - 1),
                            )
                            nc.tensor.matmul(
                                hv2[half][:, qq * QTS : (qq + 1) * QTS],
                                wv_t[:, dc, :],
                                src,
                                start=(dc == 0),
                                stop=(dc == DC - 1),
                            )
                hk_sb[b] = hkp.tile([128, L], F16, tag="hk", name=f"hk_sb{b}")
                hvT = stagep.tile([128, L], F32R, tag="hvt", bufs=2)
                for half in range(2):
                    nc.scalar.copy(
                        hk_sb[b][:, half * 1024 : (half + 1) * 1024], hk2[half][:]
                    )
                    nc.vector.tensor_copy(
                        hvT[:, half * 1024 : (half + 1) * 1024], hv2[half][:]
                    )

                # hv_aug: transpose hvT per k-tile; ones cols prefilled
                hv_sb[b] = hvp.tile(
                    [128, KN * HVW + 64], F16, tag="hv", name=f"hv_sb{b}"
                )
                nc.gpsimd.memset(hv_sb[b][:].bitcast(mybir.dt.uint16), 0x3C00)
                for kt in range(KN):
                    tp = psp.tile([128, 128], F32R, tag="sc")
                    nc.tensor.transpose(
                        tp[:], hvT[:, kt * KP : (kt + 1) * KP], identr_t[:]
                    )
                    o = kt * HVW
                    nc.vector.tensor_copy(hv_sb[b][:, o : o + DH], tp[:, 0:DH])
                    nc.vector.tensor_copy(
                        hv_sb[b][:, o + DH + 1 : o + 2 * DH + 1], tp[:, DH:128]
                    )

                hq2 = [
                    psp.tile([128, 1024], F32, tag="ctx", name=f"hq2_{b}_{i}")
                    for i in range(2)
                ]
                for dc in range(DC):
                    qc = stagep.tile([128, L], F16, tag="stage")
                    dma_split(qc, qT[b, dc], 2)
                    for half in range(2):
                        for qq in range(2):
                            nc.tensor.matmul(
                                hq2[half][:, qq * QTS : (qq + 1) * QTS],
                                wq_t[:, dc, :],
                                qc[:, (half * 2 + qq) * QTS : (half * 2 + qq + 1) * QTS],
                                start=(dc == 0),
                                stop=(dc == DC - 1),
                            )
                hq_sb[b] = hqp.tile([128, L], F16, tag="hq", name=f"hq_sb{b}")
                for half in range(2):
                    nc.scalar.copy(
                        hq_sb[b][:, half * 1024 : (half + 1) * 1024], hq2[half][:]
                    )
                if b == 0:
                    # queue the next batch's kv chunks behind the qT stream
                    # so its hk/hv phase starts with data resident
                    for dc in range(DC):
                        t = stagep.tile([128, L], F16, tag="stage", name=f"kcb1_{dc}")
                        dma_split(t, kvT[1, dc], 2)
                        kc_pre[1, dc] = t

            # ---- E stream prefetch bookkeeping ----
            groups = [
                (qt, b, kg) for qt in range(QN) for b in range(B) for kg in range(KGN)
            ]
            e_tiles = {}

            def ensure_e(gi):
                if gi >= len(groups) or gi in e_tiles:
                    return
                qt, b, kg = groups[gi]
                t = epool.tile(
                    [128, KTG, HPC * QTS], F16, tag="e", name=f"e_{qt}_{b}_{kg}"
                )
                for ki in range(KTG):
                    nc.gpsimd.dma_start(t[:, ki], eb[qt, b, kg][:, ki])
                e_tiles[gi] = t

            ensure_e(0)
            ensure_e(1)

            # ---- per-group epilogue: normalization only ----
            # The out-projection is batched into a tail phase after the
            # main loop, so the steady state carries no PSUM copies, no
            # out-proj matmuls and no out DMAs — Scalar runs pure Exp.
            class Epi:
                def __init__(self, qt, b, ctx2):
                    self.qt, self.b, self.ctx2 = qt, b, ctx2
                    self.bc = None
                    self.ctxn = None

            done_groups = []  # Epi with ctxn ready, for the tail out-proj

            def epi_step(st, step):
                qt, b, ctx2 = st.qt, st.b, st.ctx2
                if step == 0:
                    st.dsb = rcp.tile([1, 1024], F32, tag="dsb", name=f"dsb{b}_{qt}")
                    nc.vector.tensor_copy(st.dsb[:], ctx2[DH : DH + 1, :])
                elif step == 1:
                    st.rcf = rcp.tile([1, 1024], F32, tag="rcf", name=f"rcf{b}_{qt}")
                    nc.vector.reciprocal_approx_fast(st.rcf[:], st.dsb[:])
                elif step == 2:
                    st.rcr = rcp.tile([1, 1024], F16, tag="rcr", name=f"rcr{b}_{qt}")
                    nc.vector.tensor_copy(st.rcr[:], st.rcf[:])
                elif step == 3:
                    bc = rcp.tile([128, 1024], F16, tag="bcsb", name=f"bc{b}_{qt}")
                    nc.gpsimd.partition_broadcast(bc[:], st.rcr[:])
                    st.bc = bc
                else:
                    h = step - 4
                    if h == 0:
                        st.ctxn = ctxnp.tile(
                            [128, QTS], F16, tag="ctxn", bufs=8, name=f"ctxn{b}_{qt}"
                        )
                    nc.vector.tensor_tensor(
                        st.ctxn[h * DH : (h + 1) * DH, :],
                        ctx2[0:DH, h * QTS : (h + 1) * QTS],
                        st.bc[h * DH : (h + 1) * DH, h * QTS : (h + 1) * QTS],
                        mybir.AluOpType.mult,
                    )
                    if h == HPC - 1:
                        done_groups.append(st)

            EPI_AT = {3: [0], 5: [1], 7: [2], 9: [3], 13: [4], 15: [5]}

            # ---- main loop ----
            def emit_pv(b, kt, p3, pe, ctx2):
                for h in range(HPC):
                    o = kt * HVW + h * (DH + 1)
                    nc.tensor.matmul(
                        ctx2[:, h * QTS : (h + 1) * QTS],
                        hv_sb[b][:, o : o + 128],
                        p3[:, pe, h * QTS : (h + 1) * QTS],
                        start=(kt == 0),
                        stop=(kt == KN - 1),
                    )

            pending_pv = []
            cur_epi = None  # Epi of the previous (qt, b), staged into this group
            for qt in range(QN):
                for b in range(B):
                    ctx2 = psp.tile(
                        [128, 1024], F32, tag="ctx", name=f"ctx2_{qt}_{b}"
                    )
                    for kg in range(KGN):
                        gi = (qt * B + b) * KGN + kg
                        ensure_e(gi + 1)
                        ensure_e(gi + 2)
                        e4 = e_tiles[gi]
                        p2 = None
                        for ki in range(KTG):
                            kt = kg * KTG + ki
                            unit = kg * KTG + ki  # unit index within (qt, b)
                            sc2 = psp.tile(
                                [128, 1024], F32, tag="sc", name=f"sc{qt}_{b}_{kt}"
                            )
                            for h in range(HPC):
                                nc.tensor.matmul(
                                    sc2[:, h * QTS : (h + 1) * QTS],
                                    hk_sb[b][
                                        h * DH : (h + 1) * DH, kt * KP : (kt + 1) * KP
                                    ],
                                    hq_sb[b][
                                        h * DH : (h + 1) * DH,
                                        qt * QTS : (qt + 1) * QTS,
                                    ],
                                    start=True,
                                    stop=True,
                                )
                            if len(pending_pv) >= 4:
                                emit_pv(*pending_pv.pop(0))
                            if ki % 2 == 0:
                                p2 = p2p.tile(
                                    [128, 2, 1024], F16, tag="p2", name=f"p2_{qt}_{b}_{kt}"
                                )
                            nc.scalar.activation(
                                p2[:, ki % 2], sc2[:], mybir.ActivationFunctionType.Exp
                            )
                            if ki % 2 == 1:
                                # one paired e*E multiply per 2 units: 2048-wide
                                # fp16 2x-mode amortizes the DVE fixed overhead
                                p3 = p3p.tile(
                                    [128, 2, 1024], F16, tag="p3", name=f"p3_{qt}_{b}_{kt}"
                                )
                                nc.vector.tensor_tensor(
                                    p3[:], p2[:], e4[:, ki - 1 : ki + 1],
                                    mybir.AluOpType.mult,
                                )
                                pending_pv.append((b, kt - 1, p3, 0, ctx2))
                                pending_pv.append((b, kt, p3, 1, ctx2))
                                while len(pending_pv) > 4:
                                    emit_pv(*pending_pv.pop(0))
                            if cur_epi is not None and unit in EPI_AT:
                                for st in EPI_AT[unit]:
                                    epi_step(cur_epi, st)
                    cur_epi = Epi(qt, b, ctx2)
            for item in pending_pv:
                emit_pv(*item)
            for st in range(6):
                epi_step(cur_epi, st)

            # ---- tail: batched output projection over all (qt, b) ----
            # PSUM is free now: rounds rotate through sc and ctx slots (4 x
            # 2-bank tiles in flight), copies alternate Scalar/Vector.
            rounds = [
                (st, qs) for st in done_groups for qs in range(QTS // 128)
            ]
            for i, (st, qs) in enumerate(rounds):
                tag = "sc" if i % 2 == 0 else "ctx"
                op2 = psp.tile(
                    [128, 1024], F32, tag=tag, name=f"op_{st.qt}_{st.b}_{qs}"
                )
                for oh in range(2):
                    nc.tensor.matmul(
                        op2[:, oh * QTS : (oh + 1) * QTS],
                        st.ctxn[:, qs * 128 : (qs + 1) * 128],
                        wo_t[:, oh * QTS : (oh + 1) * QTS],
                        start=True,
                        stop=True,
                    )
                ob = outp.tile(
                    [128, D], F16, tag="outb", bufs=6, name=f"ob_{st.qt}_{st.b}_{qs}"
                )
                if i % 2 == 0:
                    nc.scalar.copy(ob[:], op2[:])
                else:
                    nc.vector.tensor_copy(ob[:], op2[:])
                r0 = st.qt * QTS + qs * 128
                (nc.sync if i % 2 == 0 else nc.gpsimd).dma_start(
                    out[st.b, r0 : r0 + 128, :], ob[:]
                )

    nc.compile()
    _CACHE["nc"] = nc
    return nc


def _prep_core(core, query, key_value, mask, position_bias, Wq, Wk, Wv, Wo, shared):
    """Per-core input map. `shared` holds core-independent packed arrays."""
    h0 = core * HPC
    rows = slice(h0 * DH, (h0 + HPC) * DH)

    def packw(w, scale=1.0):
        return np.ascontiguousarray(
            (w[rows].T * scale).reshape(DC, 128, 128).transpose(1, 0, 2)
        ).astype(np.float16)

    # E = mask ? exp(position_bias) : 0, packed [qt, b, kg, kp, ki, h, qf]
    expb = np.exp(position_bias[h0 : h0 + HPC], dtype=np.float32)  # [2, q, k]
    ec = (expb[None, :, :, :] * shared["maskf"][:, None, :, :]).astype(np.float16)
    # [b, h, q, k] -> [b, h, qt, qf, kg, ki, kp]
    ec = ec.reshape(B, HPC, QN, QTS, KGN, KTG, KP)
    ep = np.ascontiguousarray(ec.transpose(2, 0, 4, 6, 5, 1, 3)).reshape(
        QN, B, KGN, KP, KTG, HPC * QTS
    )
    return {
        "qT": shared["qT"],
        "kvT": shared["kvT"],
        "identr": shared["identr"],
        "wq": packw(Wq, 1.0 / np.sqrt(DH)),
        "wk": packw(Wk),
        "wv": packw(Wv),
        "wo": np.ascontiguousarray(Wo[:, rows].T).astype(np.float16),
        "eb": ep,
    }


def _prep_shared(query, key_value, mask):
    qTp = np.ascontiguousarray(
        query.reshape(B, L, DC, 128).transpose(0, 2, 3, 1)
    ).astype(np.float16)
    kvTp = np.ascontiguousarray(
        key_value.reshape(B, L, DC, 128).transpose(0, 2, 3, 1)
    ).astype(np.float16)
    return {
        "qT": qTp,
        "kvT": kvTp,
        "maskf": np.asarray(mask, dtype=bool).astype(np.float32),
        "identr": np.eye(128, dtype=np.float32),
    }


def kernel(query, key_value, mask, position_bias, Wq, Wk, Wv, Wo, _trace=False):
    query = np.asarray(query, dtype=np.float32)
    key_value = np.asarray(key_value, dtype=np.float32)
    mask = np.asarray(mask)
    position_bias = np.asarray(position_bias, dtype=np.float32)
    Wq = np.asarray(Wq, dtype=np.float32)
    Wk = np.asarray(Wk, dtype=np.float32)
    Wv = np.asarray(Wv, dtype=np.float32)
    Wo = np.asarray(Wo, dtype=np.float32)

    nc = _build()
    shared = _prep_shared(query, key_value, mask)
    in_maps = [
        _prep_core(c, query, key_value, mask, position_bias, Wq, Wk, Wv, Wo, shared)
        for c in range(N_CORES)
    ]
    res = run_bass_kernel_spmd(nc, in_maps, list(range(N_CORES)), trace=_trace)
    _CACHE["last_result"] = res
    acc = res.results[0]["out"].astype(np.float64)
    for c in range(1, N_CORES):
        acc += res.results[c]["out"]
    return acc.astype(np.float32)



# revision 7
# speedup vs baseline: 1.2243x; 1.2243x over previous
"""CPM3 attention kernel for 8 trn2 NeuronCores — v4.

Sharding: batch x heads (4 cores per batch, 4 heads per core, as two
head-pairs). Halves q/kv/out DMA vs pure head sharding. Host sums the
4 per-batch partial outputs (Wo row-sharded over the 4 head groups).

Design:
- host precomputes E = mask ? exp(position_bias) : 0 (fp16):
  softmax(s + pb - inf*mask) uses exp(s + pb)*mask = exp(s) * E.
- main loop per 128k x 1024(2 heads x 512q) tile: QK matmul (fp16 PSUM,
  1 bank) -> Exp on Scalar (the only Scalar work) -> p = e*E (fp16 DVE
  2x, paired) -> PV into fp32 ctx PSUM with a ones-column denominator.
- E streams as two [128,2048] half-tiles per group on the gpsimd+sync
  rings (1MB/group), prefetched 2 groups ahead.
- prologue: weight-stationary projection loops (LDWEIGHTS amortized
  over 4 moving pieces), copies on Scalar (idle during prologue).
- epilogue per group: reciprocal straight off the ctx PSUM denominator
  row, gpsimd partition_broadcast, per-head normalize; out-projection
  rounds interleaved into later groups' units (op PSUM tag, 2 banks).
"""

import sys

sys.path.insert(0, "/opt/trn_rl_repo")

import numpy as np

import concourse.bass as bass
import concourse.bacc as bacc
import concourse.tile as tile
import concourse.mybir as mybir
from concourse.bass_utils import run_bass_kernel_spmd

B, L, D, H, DH = 2, 2048, 1024, 16, 64
N_CORES = 8
CPB = 4  # cores per batch
HPC = 4  # heads per core
HP = 2  # head pairs per core
QTS = 512  # q tile size
QN = L // QTS  # 4
KP = 128  # k partition tile
KN = L // KP  # 16
KTG = 4  # k tiles per DMA group
KGN = KN // KTG  # 4
DC = D // 128  # 8 contraction chunks
HVW = 2 * (DH + 1)  # 130: hv_aug columns per k-tile (2 heads x (64+ones))

F32 = mybir.dt.float32
F32R = mybir.dt.float32r
F16 = mybir.dt.float16

_CACHE: dict = {}


def _build():
    if "nc" in _CACHE:
        return _CACHE["nc"]
    nc = bacc.Bacc("TRN2", target_bir_lowering=False, debug=False, num_devices=N_CORES)

    qT = nc.dram_tensor("qT", [DC, 128, L], F16, kind="ExternalInput").ap()
    kvT = nc.dram_tensor("kvT", [DC, 128, L], F16, kind="ExternalInput").ap()
    wq = nc.dram_tensor("wq", [128, HP, DC, 128], F16, kind="ExternalInput").ap()
    wk = nc.dram_tensor("wk", [128, HP, DC, 128], F16, kind="ExternalInput").ap()
    wv = nc.dram_tensor("wv", [128, HP, DC, 128], F16, kind="ExternalInput").ap()
    wo = nc.dram_tensor("wo", [128, HP, D], F16, kind="ExternalInput").ap()
    eb = nc.dram_tensor(
        "eb", [QN, HP, KGN, 128, KTG * 2 * QTS], F16, kind="ExternalInput"
    ).ap()
    identr = nc.dram_tensor("identr", [128, 128], F32R, kind="ExternalInput").ap()
    out = nc.dram_tensor("out", [L, D], F16, kind="ExternalOutput").ap()

    with tile.TileContext(nc) as tc:
        with (
            tc.tile_pool(name="const", bufs=1) as constp,
            tc.tile_pool(name="hq", bufs=2) as hqp,
            tc.tile_pool(name="hk", bufs=2) as hkp,
            tc.tile_pool(name="hv", bufs=2) as hvp,
            tc.tile_pool(name="stage", bufs=8) as stagep,
            tc.tile_pool(name="ep", bufs=3) as epool,
            tc.tile_pool(name="p2", bufs=3) as p2p,
            tc.tile_pool(name="p3", bufs=4) as p3p,
            tc.tile_pool(name="ctxn", bufs=4) as ctxnp,
            tc.tile_pool(name="rc", bufs=2) as rcp,
            tc.tile_pool(name="outb", bufs=4) as outp,
            tc.tile_pool(name="psum", bufs=2, space=bass.MemorySpace.PSUM) as psp,
        ):
            # ---- constants (sync ring) ----
            identr_t = constp.tile([128, 128], F32R, tag="identr")
            nc.sync.dma_start(identr_t[:], identr[:])
            wq_t = constp.tile([128, HP, DC, 128], F16, tag="wq")
            nc.sync.dma_start(wq_t[:], wq[:])
            wk_t = constp.tile([128, HP, DC, 128], F16, tag="wk")
            nc.sync.dma_start(wk_t[:], wk[:])
            wv_t = constp.tile([128, HP, DC, 128], F16, tag="wv")
            nc.sync.dma_start(wv_t[:], wv[:])
            wo_t = constp.tile([128, HP, D], F16, tag="wo")
            nc.sync.dma_start(wo_t[:], wo[:])

            # DMA triggers cost ~650ns on the issuing engine queue. Prologue
            # chunk loads cycle sync/gpsimd/scalar (all prologue-only work on
            # scalar finishes before the first Exp enters its FIFO).
            trig = [nc.sync, nc.gpsimd, nc.scalar]
            trig_i = [0]

            def dma_split(dst, src, n):
                w = L // n
                for s in range(n):
                    eng = trig[trig_i[0] % 3]
                    trig_i[0] += 1
                    eng.dma_start(
                        dst[:, s * w : (s + 1) * w], src[:, s * w : (s + 1) * w]
                    )

            # ---- prologue: kv chunks stream once; hk+hv for both head
            # pairs via weight-stationary loops; then q stream + hq.
            kc = {}
            for dc in range(DC):
                kc[dc] = stagep.tile([128, L], F16, tag="stage", name=f"kc{dc}")
                dma_split(kc[dc], kvT[dc], 4 if dc < 2 else 2)

            hk_sb, hq_sb, hv_sb = {}, {}, {}
            hvT = {}
            for hp in range(HP):
                hk2 = [
                    psp.tile([128, 1024], F32, tag="ctx", name=f"hk2_{hp}_{i}")
                    for i in range(2)
                ]
                hv2 = [
                    psp.tile([128, 1024], F32, tag="sc", name=f"hv2_{hp}_{i}")
                    for i in range(2)
                ]
                for dc in range(DC):
                    st, sp = (dc == 0), (dc == DC - 1)
                    for p in range(4):
                        nc.tensor.matmul(
                            hk2[p // 2][:, (p % 2) * 512 : (p % 2 + 1) * 512],
                            wk_t[:, hp, dc, :],
                            kc[dc][:, p * 512 : (p + 1) * 512],
                            start=st,
                            stop=sp,
                        )
                    for p in range(4):
                        nc.tensor.matmul(
                            hv2[p // 2][:, (p % 2) * 512 : (p % 2 + 1) * 512],
                            wv_t[:, hp, dc, :],
                            kc[dc][:, p * 512 : (p + 1) * 512],
                            start=st,
                            stop=sp,
                        )
                hk_sb[hp] = hkp.tile([128, L], F16, tag="hk", name=f"hk_sb{hp}")
                for i in range(2):
                    nc.scalar.copy(
                        hk_sb[hp][:, i * 1024 : (i + 1) * 1024], hk2[i][:]
                    )
                hvT[hp] = stagep.tile(
                    [128, L], F32R, tag="hvt", bufs=2, name=f"hvT{hp}"
                )
                nc.vector.tensor_copy(hvT[hp][:, 0:1024], hv2[0][:])
                nc.vector.tensor_copy(hvT[hp][:, 1024:2048], hv2[1][:])

                # hv_aug: transpose hvT per k-tile; ones cols prefilled
                hv_sb[hp] = hvp.tile(
                    [128, KN * HVW + 64], F16, tag="hv", name=f"hv_sb{hp}"
                )
                nc.gpsimd.memset(hv_sb[hp][:].bitcast(mybir.dt.uint16), 0x3C00)
                for kt in range(KN):
                    tp = psp.tile([128, 128], F32R, tag="sc", name=f"tp{hp}_{kt}")
                    nc.tensor.transpose(
                        tp[:], hvT[hp][:, kt * KP : (kt + 1) * KP], identr_t[:]
                    )
                    o = kt * HVW
                    nc.vector.tensor_copy(hv_sb[hp][:, o : o + DH], tp[:, 0:DH])
                    nc.vector.tensor_copy(
                        hv_sb[hp][:, o + DH + 1 : o + 2 * DH + 1], tp[:, DH:128]
                    )

            qc = {}
            for dc in range(DC):
                qc[dc] = stagep.tile([128, L], F16, tag="stage", name=f"qc{dc}")
                dma_split(qc[dc], qT[dc], 2)
            for hp in range(HP):
                hq2 = [
                    psp.tile([128, 1024], F32, tag="ctx", name=f"hq2_{hp}_{i}")
                    for i in range(2)
                ]
                for dc in range(DC):
                    for p in range(4):
                        nc.tensor.matmul(
                            hq2[p // 2][:, (p % 2) * 512 : (p % 2 + 1) * 512],
                            wq_t[:, hp, dc, :],
                            qc[dc][:, p * 512 : (p + 1) * 512],
                            start=(dc == 0),
                            stop=(dc == DC - 1),
                        )
                hq_sb[hp] = hqp.tile([128, L], F16, tag="hq", name=f"hq_sb{hp}")
                for i in range(2):
                    nc.scalar.copy(
                        hq_sb[hp][:, i * 1024 : (i + 1) * 1024], hq2[i][:]
                    )

            # ---- E stream prefetch bookkeeping ----
            egroups = [
                (qt, hp, kg)
                for qt in range(QN)
                for hp in range(HP)
                for kg in range(KGN)
            ]
            e_tiles = {}

            def ensure_e(gi):
                if gi >= len(egroups) or gi in e_tiles:
                    return
                qt, hp, kg = egroups[gi]
                t = epool.tile(
                    [128, KTG, 2 * QTS], F16, tag="e", name=f"e_{qt}_{hp}_{kg}"
                )
                src = eb[qt, hp, kg]
                nc.gpsimd.dma_start(t[:, 0:2], src[:, 0:2048])
                nc.sync.dma_start(t[:, 2:4], src[:, 2048:4096])
                e_tiles[gi] = t

            ensure_e(0)
            ensure_e(1)

            # ---- per-group epilogue: normalization only ----
            class Epi:
                def __init__(self, qt, hp, ctx2):
                    self.qt, self.hp, self.ctx2 = qt, hp, ctx2
                    self.bc = None
                    self.ctxn = None

            ctxn_done = {}  # (qt, hp) -> ctxn tile
            pending_tail = []

            def epi_step(st, step):
                qt, hp, ctx2 = st.qt, st.hp, st.ctx2
                if step == 0:
                    st.dsb = rcp.tile([1, 1024], F32, tag="dsb", name=f"dsb{hp}_{qt}")
                    nc.vector.tensor_copy(st.dsb[:], ctx2[DH : DH + 1, :])
                elif step == 1:
                    st.rcf = rcp.tile([1, 1024], F32, tag="rcf", name=f"rcf{hp}_{qt}")
                    nc.vector.reciprocal_approx_fast(st.rcf[:], st.dsb[:])
                elif step == 2:
                    st.rcr = rcp.tile([1, 1024], F16, tag="rcr", name=f"rcr{hp}_{qt}")
                    nc.vector.tensor_copy(st.rcr[:], st.rcf[:])
                elif step == 3:
                    bc = rcp.tile([128, 1024], F16, tag="bcsb", name=f"bc{hp}_{qt}")
                    nc.gpsimd.partition_broadcast(bc[:], st.rcr[:])
                    st.bc = bc
                else:
                    h = step - 4
                    if h == 0:
                        st.ctxn = ctxnp.tile(
                            [128, QTS], F16, tag="ctxn", bufs=8, name=f"ctxn{hp}_{qt}"
                        )
                    nc.vector.tensor_tensor(
                        st.ctxn[h * DH : (h + 1) * DH, :],
                        ctx2[0:DH, h * QTS : (h + 1) * QTS],
                        st.bc[h * DH : (h + 1) * DH, h * QTS : (h + 1) * QTS],
                        mybir.AluOpType.mult,
                    )
                    if h == 1:
                        ctxn_done[qt, hp] = st.ctxn
                        if hp == 1:
                            for qs in range(4):
                                pending_tail.append((qt, qs))

            EPI_AT = {3: [0], 5: [1], 7: [2], 9: [3], 11: [4], 13: [5]}

            # ---- out-projection tail round (batched after the main loop) ----
            tail_i = [0]

            def emit_tail(qt, qs):
                i = tail_i[0]
                tail_i[0] += 1
                op2 = psp.tile(
                    [128, 1024], F32, tag="sc" if i % 2 == 0 else "ctx",
                    name=f"op_{qt}_{qs}"
                )
                for hp in range(HP):
                    for oh in range(2):
                        nc.tensor.matmul(
                            op2[:, oh * 512 : (oh + 1) * 512],
                            ctxn_done[qt, hp][:, qs * 128 : (qs + 1) * 128],
                            wo_t[:, hp, oh * 512 : (oh + 1) * 512],
                            start=(hp == 0),
                            stop=(hp == 1),
                        )
                ob = outp.tile([128, D], F16, tag="outb", bufs=6, name=f"ob_{qt}_{qs}")
                if i % 2 == 0:
                    nc.scalar.copy(ob[:], op2[:])
                else:
                    nc.vector.tensor_copy(ob[:], op2[:])
                r0 = qt * QTS + qs * 128
                eng = nc.sync if i % 2 == 0 else nc.gpsimd
                eng.dma_start(out[r0 : r0 + 128, :], ob[:])

            # ---- main loop ----
            def emit_pv(hp, kt, p3, pe, ctx2):
                for h in range(2):
                    o = kt * HVW + h * (DH + 1)
                    nc.tensor.matmul(
                        ctx2[:, h * QTS : (h + 1) * QTS],
                        hv_sb[hp][:, o : o + 128],
                        p3[:, pe, h * QTS : (h + 1) * QTS],
                        start=(kt == 0),
                        stop=(kt == KN - 1),
                    )

            pending_pv = []
            cur_epi = None
            for qt in range(QN):
                for hp in range(HP):
                    ctx2 = psp.tile(
                        [128, 1024], F32, tag="ctx", name=f"ctx2_{qt}_{hp}"
                    )
                    for kg in range(KGN):
                        gi = (qt * HP + hp) * KGN + kg
                        ensure_e(gi + 1)
                        ensure_e(gi + 2)
                        e4 = e_tiles[gi]
                        p2 = None
                        for ki in range(KTG):
                            kt = kg * KTG + ki
                            unit = kg * KTG + ki
                            sc2 = psp.tile(
                                [128, 1024], F32, tag="sc", name=f"sc{qt}_{hp}_{kt}"
                            )
                            for h in range(2):
                                nc.tensor.matmul(
                                    sc2[:, h * QTS : (h + 1) * QTS],
                                    hk_sb[hp][
                                        h * DH : (h + 1) * DH, kt * KP : (kt + 1) * KP
                                    ],
                                    hq_sb[hp][
                                        h * DH : (h + 1) * DH,
                                        qt * QTS : (qt + 1) * QTS,
                                    ],
                                    start=True,
                                    stop=True,
                                )
                            if len(pending_pv) >= 4:
                                emit_pv(*pending_pv.pop(0))
                            if ki % 2 == 0:
                                p2 = p2p.tile(
                                    [128, 2, 1024], F16, tag="p2", name=f"p2_{qt}_{hp}_{kt}"
                                )
                            nc.scalar.activation(
                                p2[:, ki % 2], sc2[:], mybir.ActivationFunctionType.Exp
                            )
                            if ki % 2 == 1:
                                p3 = p3p.tile(
                                    [128, 2, 1024], F16, tag="p3", name=f"p3_{qt}_{hp}_{kt}"
                                )
                                nc.vector.tensor_tensor(
                                    p3[:], p2[:], e4[:, ki - 1 : ki + 1],
                                    mybir.AluOpType.mult,
                                )
                                pending_pv.append((hp, kt - 1, p3, 0, ctx2))
                                pending_pv.append((hp, kt, p3, 1, ctx2))
                                while len(pending_pv) > 4:
                                    emit_pv(*pending_pv.pop(0))
                            if cur_epi is not None and unit in EPI_AT:
                                for s in EPI_AT[unit]:
                                    epi_step(cur_epi, s)
                    cur_epi = Epi(qt, hp, ctx2)
            for item in pending_pv:
                emit_pv(*item)
            for s in range(6):
                epi_step(cur_epi, s)
            while pending_tail:
                emit_tail(*pending_tail.pop(0))

    nc.compile()
    _CACHE["nc"] = nc
    return nc


def _prep_core(core, position_bias, Wq, Wk, Wv, Wo, shared):
    bc = core // CPB
    h0 = (core % CPB) * HPC
    rows = slice(h0 * DH, (h0 + HPC) * DH)  # 256 rows

    def packw(w, scale=1.0):
        wr = w[rows].T * scale  # [D, 256]
        return np.ascontiguousarray(
            np.stack(
                [
                    wr[:, hp * 128 : (hp + 1) * 128]
                    .reshape(DC, 128, 128)
                    .transpose(1, 0, 2)
                    for hp in range(HP)
                ]
            ).transpose(1, 0, 2, 3)
        ).astype(np.float16)

    # E = mask ? exp(position_bias) : 0, packed [qt, hp, kg, kp, (ki h2 qf)]
    expb = np.exp(position_bias[h0 : h0 + HPC], dtype=np.float32)  # [4, q, k]
    ec = (expb * shared["maskf"][bc][None]).astype(np.float16)
    ec = ec.reshape(HP, 2, QN, QTS, KGN, KTG, KP)
    ep = np.ascontiguousarray(ec.transpose(2, 0, 4, 6, 5, 1, 3)).reshape(
        QN, HP, KGN, KP, KTG * 2 * QTS
    )
    wor = Wo[:, rows].T  # [256, D]
    return {
        "qT": shared["qT"][bc],
        "kvT": shared["kvT"][bc],
        "identr": shared["identr"],
        "wq": packw(Wq, 1.0 / np.sqrt(DH)),
        "wk": packw(Wk),
        "wv": packw(Wv),
        "wo": np.ascontiguousarray(
            np.stack([wor[hp * 128 : (hp + 1) * 128] for hp in range(HP)]).transpose(
                1, 0, 2
            )
        ).astype(np.float16),
        "eb": ep,
    }


def _prep_shared(query, key_value, mask):
    qTp = np.ascontiguousarray(
        query.reshape(B, L, DC, 128).transpose(0, 2, 3, 1)
    ).astype(np.float16)
    kvTp = np.ascontiguousarray(
        key_value.reshape(B, L, DC, 128).transpose(0, 2, 3, 1)
    ).astype(np.float16)
    return {
        "qT": qTp,
        "kvT": kvTp,
        "maskf": np.asarray(mask, dtype=bool).astype(np.float32),
        "identr": np.eye(128, dtype=np.float32),
    }


def kernel(query, key_value, mask, position_bias, Wq, Wk, Wv, Wo, _trace=False):
    query = np.asarray(query, dtype=np.float32)
    key_value = np.asarray(key_value, dtype=np.float32)
    mask = np.asarray(mask)
    position_bias = np.asarray(position_bias, dtype=np.float32)
    Wq = np.asarray(Wq, dtype=np.float32)
    Wk = np.asarray(Wk, dtype=np.float32)
    Wv = np.asarray(Wv, dtype=np.float32)
    Wo = np.asarray(Wo, dtype=np.float32)

    nc = _build()
    shared = _prep_shared(query, key_value, mask)
    in_maps = [
        _prep_core(c, position_bias, Wq, Wk, Wv, Wo, shared) for c in range(N_CORES)
    ]
    res = run_bass_kernel_spmd(nc, in_maps, list(range(N_CORES)), trace=_trace)
    _CACHE["last_result"] = res
    full = np.zeros((B, L, D), np.float64)
    for c in range(N_CORES):
        full[c // CPB] += res.results[c]["out"].astype(np.float64)
    return full.astype(np.float32)
